# revision 7
# baseline (speedup 1.0000x reference)
"""Trainium2 Bass kernel for single-"head" LlamaAttention.

Reference computation (per batch b):
    q = hs @ Wq.T ; k = hs @ Wk.T ; v = hs @ Wv.T          # [S, H]
    scores = (q @ k.T) / sqrt(128) + mask                  # [S, S]
    probs  = softmax(scores, axis=-1)
    out    = (probs @ v) @ Wo.T                            # [S, H]

Shapes: B=2, S=4096, H=2048, fp32 I/O.

Sharding: 8 cores, 4 per batch element; each core owns 1024 query rows and
computes K/V for its whole batch locally (replicated within the 4-core
group), so no collectives are needed.  Key order is rotated per core so the
core's query columns are always cols [0:QPC) of its (transposed) hidden
input — softmax over keys is permutation invariant as long as K, V and the
mask columns use the same order.

Per-core pipeline (all matmuls bf16 with fp32 PSUM accumulation):
  A:   QT[h,q]   = WqT.T-chunks @ hsT[:, :QPC]     -> qt_d (DRAM scratch)
  B:   KT[h,k]   = Wk proj                          -> kt_d
  C:   V[k,h]    = Wv proj                          -> v_d
  D (per query half):
    D1:  S^T[k,q] = KT-chunks.T @ QT-chunks  (keys on partitions)
         P^T = exp(scale*S^T + mask^T)  (ACT), l[q] = ones.T @ P^T (PE)
    D2:  rl = 1/l (DVE), broadcast to 128 partitions via K=1 matmul
    D3:  ctx^T[h,q] = sum_k V-tiles.T @ P^T, multiplied by rl on PSUM->SBUF
    D4:  O[q,h] = ctx^T-chunks.T @ WoT  -> output
"""

import math
import os
import sys

import numpy as np

sys.path.insert(0, "/opt/trn_rl_repo")

import ml_dtypes  # noqa: E402

import concourse.bass as bass  # noqa: E402
import concourse.tile as tile  # noqa: E402
from concourse import bacc, mybir  # noqa: E402
from concourse.bass_utils import run_bass_kernel_spmd  # noqa: E402

BF16 = mybir.dt.bfloat16
F32 = mybir.dt.float32
NP_BF16 = ml_dtypes.bfloat16


class Cfg:
    def __init__(self, S=4096, H=2048, QPC=1024, head_dim=128):
        self.S = S          # keys per batch
        self.H = H          # hidden
        self.QPC = QPC      # queries per core
        self.HC = H // 128  # contraction chunks
        self.KB = S // 128  # key blocks
        self.NH = 512       # free-dim tile
        self.QR = min(QPC, 512)  # queries per round
        self.NQG = QPC // self.QR
        self.scale = 1.0 / math.sqrt(head_dim)


def build_nc(cfg: Cfg) -> bass.Bass:
    S, H, QPC = cfg.S, cfg.H, cfg.QPC
    HC, KB, NH, QR, NQG = cfg.HC, cfg.KB, cfg.NH, cfg.QR, cfg.NQG

    nc = bacc.Bacc(None, target_bir_lowering=False, num_devices=8)

    hsT = nc.dram_tensor("hst", [H, S], BF16, kind="ExternalInput")
    wqT = nc.dram_tensor("wqt", [H, H], BF16, kind="ExternalInput")
    wkT = nc.dram_tensor("wkt", [H, H], BF16, kind="ExternalInput")
    wvT = nc.dram_tensor("wvt", [H, H], BF16, kind="ExternalInput")
    woT = nc.dram_tensor("wot", [H, H], BF16, kind="ExternalInput")
    maskT = nc.dram_tensor("maskt", [S, QPC], BF16, kind="ExternalInput")
    o_out = nc.dram_tensor("o", [QPC, H], F32, kind="ExternalOutput")

    qt_d = nc.dram_tensor("qt_d", [HC, 128, QPC], BF16)
    kt_d = nc.dram_tensor("kt_d", [HC, 128, S], BF16)
    v_d = nc.dram_tensor("v_d", [S, H], BF16)

    mm = mybir.AluOpType.mult

    with tile.TileContext(nc) as tc:
        with (
            tc.tile_pool(name="w", bufs=1) as wpool,
            tc.tile_pool(name="stream", bufs=2) as spool,
            tc.tile_pool(name="qt", bufs=1) as qtpool,
            tc.tile_pool(name="pt", bufs=1) as ptpool,
            tc.tile_pool(name="ct", bufs=1) as ctpool,
            tc.tile_pool(name="mk", bufs=1) as mkpool,
            tc.tile_pool(name="v", bufs=2) as vpool,
            tc.tile_pool(name="stg", bufs=4) as stpool,
            tc.tile_pool(name="stgf", bufs=3) as stfpool,
            tc.tile_pool(name="misc", bufs=1) as mpool,
            tc.tile_pool(name="ps", bufs=8, space="PSUM") as pspool,
        ):
            # ---------------- Phase A: QT projection -------------------
            wq = wpool.tile([128, HC, H], BF16, tag="w")
            nc.sync.dma_start(out=wq[:], in_=wqT[:].rearrange("(c p) o -> p c o", p=128))
            for qg in range(NQG):
                hq = spool.tile([128, HC, QR], BF16, tag="stream")
                nc.sync.dma_start(
                    out=hq[:],
                    in_=hsT[:, qg * QR:(qg + 1) * QR].rearrange("(c p) q -> p c q", p=128),
                )
                for hb in range(HC):
                    ps = pspool.tile([128, QR], F32, tag="ps")
                    for hc in range(HC):
                        nc.tensor.matmul(
                            ps[:],
                            wq[:, hc, hb * 128:(hb + 1) * 128],
                            hq[:, hc, :],
                            start=(hc == 0),
                            stop=(hc == HC - 1),
                        )
                    st = stpool.tile([128, QR], BF16, tag="stg")
                    nc.scalar.copy(st[:], ps[:])
                    nc.sync.dma_start(
                        out=qt_d[hb, :, qg * QR:(qg + 1) * QR], in_=st[:]
                    )

            # ---------------- Phase B: KT projection -------------------
            wk = wpool.tile([128, HC, H], BF16, tag="w")
            nc.sync.dma_start(out=wk[:], in_=wkT[:].rearrange("(c p) o -> p c o", p=128))
            for kcb in range(S // NH):
                hsb = spool.tile([128, HC, NH], BF16, tag="stream")
                nc.sync.dma_start(
                    out=hsb[:],
                    in_=hsT[:, kcb * NH:(kcb + 1) * NH].rearrange("(c p) k -> p c k", p=128),
                )
                for hb in range(HC):
                    ps = pspool.tile([128, NH], F32, tag="ps")
                    for hc in range(HC):
                        nc.tensor.matmul(
                            ps[:],
                            wk[:, hc, hb * 128:(hb + 1) * 128],
                            hsb[:, hc, :],
                            start=(hc == 0),
                            stop=(hc == HC - 1),
                        )
                    st = stpool.tile([128, NH], BF16, tag="stg")
                    nc.scalar.copy(st[:], ps[:])
                    nc.sync.dma_start(
                        out=kt_d[hb, :, kcb * NH:(kcb + 1) * NH], in_=st[:]
                    )

            # ---------------- Phase C: V projection --------------------
            wv = wpool.tile([128, HC, H], BF16, tag="w")
            nc.sync.dma_start(out=wv[:], in_=wvT[:].rearrange("(c p) o -> p c o", p=128))
            for kcb in range(S // NH):
                hsb = spool.tile([128, HC, NH], BF16, tag="stream")
                nc.sync.dma_start(
                    out=hsb[:],
                    in_=hsT[:, kcb * NH:(kcb + 1) * NH].rearrange("(c p) k -> p c k", p=128),
                )
                for kb4 in range(NH // 128):
                    for hh in range(H // NH):
                        ps = pspool.tile([128, NH], F32, tag="ps")
                        for hc in range(HC):
                            nc.tensor.matmul(
                                ps[:],
                                hsb[:, hc, kb4 * 128:(kb4 + 1) * 128],
                                wv[:, hc, hh * NH:(hh + 1) * NH],
                                start=(hc == 0),
                                stop=(hc == HC - 1),
                            )
                        st = stpool.tile([128, NH], BF16, tag="stg")
                        nc.scalar.copy(st[:], ps[:])
                        nc.sync.dma_start(
                            out=v_d[
                                kcb * NH + kb4 * 128: kcb * NH + (kb4 + 1) * 128,
                                hh * NH:(hh + 1) * NH,
                            ],
                            in_=st[:],
                        )

            # ---------------- Phase D: attention + out-proj ------------
            ones_col = mpool.tile([128, 1], BF16, tag="m_ones")
            nc.vector.memset(ones_col[:], 1.0)
            ones_row = mpool.tile([1, 128], F32, tag="m_onesr")
            nc.vector.memset(ones_row[:], 1.0)

            wo = wpool.tile([128, HC, H], BF16, tag="w")
            nc.sync.dma_start(out=wo[:], in_=woT[:].rearrange("(c p) o -> p c o", p=128))

            for qg in range(NQG):
                qsl = slice(qg * QR, (qg + 1) * QR)
                qt = qtpool.tile([128, HC, QR], BF16, tag="qt")
                nc.sync.dma_start(
                    out=qt[:], in_=qt_d[:, :, qsl].rearrange("c p q -> p c q")
                )
                pt = ptpool.tile([128, KB, QR], BF16, tag="pt")
                l_ps = pspool.tile([1, QR], F32, tag="ps")

                # --- D1: S^T, P^T = exp(scale*S^T + mask^T), l ---
                # The l-accumulation matmul for key-block kb is emitted two
                # iterations late so the PE never waits on the DVE/ACT
                # pipeline that produces pt[:, kb, :].
                def emit_l_mm(kb):
                    nc.tensor.matmul(
                        l_ps[:],
                        ones_col[:, 0:1],
                        pt[:, kb, :],
                        start=(kb == 0),
                        stop=(kb == KB - 1),
                    )

                for kbq in range(KB // 4):
                    ktq = spool.tile([128, HC, NH], BF16, tag="stream")
                    nc.sync.dma_start(
                        out=ktq[:],
                        in_=kt_d[:, :, kbq * NH:(kbq + 1) * NH].rearrange("c p k -> p c k"),
                    )
                    mk = mkpool.tile([128, 4, QR], BF16, tag="mk")
                    nc.sync.dma_start(
                        out=mk[:],
                        in_=maskT[kbq * NH:(kbq + 1) * NH, qsl].rearrange(
                            "(b p) q -> p b q", p=128
                        ),
                    )
                    for kb4 in range(4):
                        kb = kbq * 4 + kb4
                        ps = pspool.tile([128, QR], F32, tag="ps")
                        for hc in range(HC):
                            nc.tensor.matmul(
                                ps[:],
                                ktq[:, hc, kb4 * 128:(kb4 + 1) * 128],
                                qt[:, hc, :],
                                start=(hc == 0),
                                stop=(hc == HC - 1),
                            )
                        tmp = stfpool.tile([128, QR], F32, tag="stgf")
                        nc.vector.scalar_tensor_tensor(
                            out=tmp[:],
                            in0=ps[:],
                            scalar=cfg.scale,
                            in1=mk[:, kb4, :],
                            op0=mm,
                            op1=mybir.AluOpType.add,
                        )
                        nc.scalar.activation(
                            out=pt[:, kb, :], in_=tmp[:],
                            func=mybir.ActivationFunctionType.Exp,
                        )
                        if kb >= 2:
                            emit_l_mm(kb - 2)
                for kb in range(KB - 2, KB):
                    emit_l_mm(kb)

                # --- D2: reciprocal + broadcast ---
                l_sb = mpool.tile([1, QR], F32, tag="m_l")
                nc.scalar.copy(l_sb[:], l_ps[:])
                rl = mpool.tile([1, QR], F32, tag="m_rl")
                nc.vector.reciprocal(rl[:], l_sb[:])
                b_ps = pspool.tile([128, QR], F32, tag="ps")
                nc.tensor.matmul(b_ps[:], ones_row[:], rl[:], start=True, stop=True)
                rb = mpool.tile([128, QR], F32, tag="m_rb")
                nc.scalar.copy(rb[:], b_ps[:])

                # --- D3: ctx^T with fused divide ---
                ct = ctpool.tile([128, HC, QR], BF16, tag="ct")
                for ho in range(HC // 8):
                    cps = [
                        pspool.tile([128, QR], F32, tag="ps", name=f"cps{i}")
                        for i in range(8)
                    ]
                    for kb in range(KB):
                        vt = vpool.tile([128, 8, 128], BF16, tag="v")
                        nc.sync.dma_start(
                            out=vt[:],
                            in_=v_d[
                                kb * 128:(kb + 1) * 128,
                                ho * 1024:(ho + 1) * 1024,
                            ].rearrange("p (b h) -> p b h", b=8),
                        )
                        for i8 in range(8):
                            nc.tensor.matmul(
                                cps[i8][:],
                                vt[:, i8, :],
                                pt[:, kb, :],
                                start=(kb == 0),
                                stop=(kb == KB - 1),
                            )
                    for i8 in range(8):
                        nc.vector.tensor_mul(
                            out=ct[:, ho * 8 + i8, :], in0=cps[i8][:], in1=rb[:]
                        )

                # --- D4: output projection ---
                for qb in range(QR // 128):
                    for hh in range(H // NH):
                        ps = pspool.tile([128, NH], F32, tag="ps")
                        for hc in range(HC):
                            nc.tensor.matmul(
                                ps[:],
                                ct[:, hc, qb * 128:(qb + 1) * 128],
                                wo[:, hc, hh * NH:(hh + 1) * NH],
                                start=(hc == 0),
                                stop=(hc == HC - 1),
                            )
                        ob = stfpool.tile([128, NH], F32, tag="stgf")
                        nc.scalar.copy(ob[:], ps[:])
                        nc.sync.dma_start(
                            out=o_out[
                                qg * QR + qb * 128: qg * QR + (qb + 1) * 128,
                                hh * NH:(hh + 1) * NH,
                            ],
                            in_=ob[:],
                        )
    nc.finalize()
    return nc


def make_in_maps(cfg: Cfg, hidden_states, attention_mask, Wq, Wk, Wv, Wo, n_cores=8):
    """Build the 8 per-core input dicts (host-side prep: transpose + bf16)."""
    B = hidden_states.shape[0]
    gpc = n_cores // B  # cores per batch element
    wq_t = np.ascontiguousarray(Wq.T.astype(NP_BF16))
    wk_t = np.ascontiguousarray(Wk.T.astype(NP_BF16))
    wv_t = np.ascontiguousarray(Wv.T.astype(NP_BF16))
    wo_t = np.ascontiguousarray(Wo.T.astype(NP_BF16))
    in_maps = []
    for c in range(n_cores):
        b, g = c // gpc, c % gpc
        q0 = g * cfg.QPC
        hsT_b = hidden_states[b].T.astype(NP_BF16)  # [H, S]
        hsT_c = np.ascontiguousarray(np.roll(hsT_b, -q0, axis=1))
        msk = attention_mask[b, q0:q0 + cfg.QPC, :]  # [QPC, S]
        mskT_c = np.ascontiguousarray(np.roll(msk, -q0, axis=1).T.astype(NP_BF16))
        in_maps.append(
            {
                "hst": hsT_c,
                "wqt": wq_t,
                "wkt": wk_t,
                "wvt": wv_t,
                "wot": wo_t,
                "maskt": mskT_c,
            }
        )
    return in_maps


def assemble_output(cfg: Cfg, results, B, S, H, n_cores=8):
    out = np.empty((B, S, H), dtype=np.float32)
    gpc = n_cores // B
    for c in range(n_cores):
        b, g = c // gpc, c % gpc
        out[b, g * cfg.QPC:(g + 1) * cfg.QPC, :] = results[c]["o"]
    return out


_CACHED_NC = None


def kernel(hidden_states, attention_mask, Wq, Wk, Wv, Wo, **kw):
    global _CACHED_NC
    B, S, H = hidden_states.shape
    cfg = Cfg(S=S, H=H, QPC=(B * S) // 8)
    if _CACHED_NC is None:
        _CACHED_NC = build_nc(cfg)
    nc = _CACHED_NC
    in_maps = make_in_maps(cfg, np.asarray(hidden_states), np.asarray(attention_mask),
                           np.asarray(Wq), np.asarray(Wk), np.asarray(Wv), np.asarray(Wo))
    core_ids = list(range(8))
    res = run_bass_kernel_spmd(nc, in_maps, core_ids)
    return assemble_output(cfg, res.results, B, S, H)


# revision 11
# speedup vs baseline: 1.0002x; 1.0002x over previous
"""Trainium2 Bass kernel for single-"head" LlamaAttention.

Reference computation (per batch b):
    q = hs @ Wq.T ; k = hs @ Wk.T ; v = hs @ Wv.T          # [S, H]
    scores = (q @ k.T) / sqrt(128) + mask                  # [S, S]
    probs  = softmax(scores, axis=-1)
    out    = (probs @ v) @ Wo.T                            # [S, H]

Shapes: B=2, S=4096, H=2048, fp32 I/O.

Sharding: 8 cores, 4 per batch element; each core owns 1024 query rows and
computes K/V for its whole batch locally (replicated within the 4-core
group), so no collectives are needed.  Key order is rotated per core so the
core's query columns are always cols [0:QPC) of its (transposed) hidden
input — softmax over keys is permutation invariant as long as K, V and the
mask columns use the same order.

Per-core pipeline (all matmuls bf16 with fp32 PSUM accumulation):
  A:   QT[h,q]   = WqT.T-chunks @ hsT[:, :QPC]     -> qt_d (DRAM scratch)
  B:   KT[h,k]   = Wk proj                          -> kt_d
  C:   V[k,h]    = Wv proj                          -> v_d
  D (per query half):
    D1:  S^T[k,q] = KT-chunks.T @ QT-chunks  (keys on partitions)
         P^T = exp(scale*S^T + mask^T)  (ACT), l[q] = ones.T @ P^T (PE)
    D2:  rl = 1/l (DVE), broadcast to 128 partitions via K=1 matmul
    D3:  ctx^T[h,q] = sum_k V-tiles.T @ P^T, multiplied by rl on PSUM->SBUF
    D4:  O[q,h] = ctx^T-chunks.T @ WoT  -> output
"""

import math
import os
import sys

import numpy as np

sys.path.insert(0, "/opt/trn_rl_repo")

import ml_dtypes  # noqa: E402

import concourse.bass as bass  # noqa: E402
import concourse.tile as tile  # noqa: E402
from concourse import bacc, mybir  # noqa: E402
from concourse.bass_utils import run_bass_kernel_spmd  # noqa: E402

BF16 = mybir.dt.bfloat16
F32 = mybir.dt.float32
NP_BF16 = ml_dtypes.bfloat16


class Cfg:
    def __init__(self, S=4096, H=2048, QPC=1024, head_dim=128):
        self.S = S          # keys per batch
        self.H = H          # hidden
        self.QPC = QPC      # queries per core (also keys per core shard)
        self.GPC = 4        # cores per batch group
        self.HC = H // 128  # contraction chunks
        self.KB = S // 128  # key blocks
        self.NH = 512       # free-dim tile
        self.QR = min(QPC, 512)  # queries per round
        self.NQG = QPC // self.QR
        self.scale = 1.0 / math.sqrt(head_dim)
        assert QPC % self.NH == 0 and S == self.GPC * QPC


def build_nc(cfg: Cfg) -> bass.Bass:
    S, H, QPC = cfg.S, cfg.H, cfg.QPC
    HC, KB, NH, QR, NQG, GPC = cfg.HC, cfg.KB, cfg.NH, cfg.QR, cfg.NQG, cfg.GPC

    nc = bacc.Bacc(None, target_bir_lowering=False, num_devices=2 * GPC)

    hsq = nc.dram_tensor("hsq", [H, QPC], BF16, kind="ExternalInput")
    wqT = nc.dram_tensor("wqt", [H, H], BF16, kind="ExternalInput")
    wkT = nc.dram_tensor("wkt", [H, H], BF16, kind="ExternalInput")
    wvT = nc.dram_tensor("wvt", [H, H], BF16, kind="ExternalInput")
    woT = nc.dram_tensor("wot", [H, H], BF16, kind="ExternalInput")
    maskT = nc.dram_tensor("maskt", [S, QPC], BF16, kind="ExternalInput")
    o_out = nc.dram_tensor("o", [QPC, H], F32, kind="ExternalOutput")

    qt_d = nc.dram_tensor("qt_d", [HC, 128, QPC], BF16)
    kt_p = nc.dram_tensor("kt_p", [HC, 128, QPC], BF16)
    v_p = nc.dram_tensor("v_p", [QPC, H], BF16)
    kt_g = nc.dram_tensor("kt_g", [GPC, HC, 128, QPC], BF16)
    v_g = nc.dram_tensor("v_g", [GPC, QPC, H], BF16)

    groups = [list(range(g * GPC, (g + 1) * GPC)) for g in range(2)]
    mm = mybir.AluOpType.mult

    with tile.TileContext(nc) as tc:
        with (
            tc.tile_pool(name="hs", bufs=1) as hpool,
            tc.tile_pool(name="w", bufs=1) as wpool,
            tc.tile_pool(name="stream", bufs=3) as spool,
            tc.tile_pool(name="qt", bufs=1) as qtpool,
            tc.tile_pool(name="pt", bufs=1) as ptpool,
            tc.tile_pool(name="ct", bufs=1) as ctpool,
            tc.tile_pool(name="mk", bufs=1) as mkpool,
            tc.tile_pool(name="v", bufs=2) as vpool,
            tc.tile_pool(name="stg", bufs=3) as stpool,
            tc.tile_pool(name="stgf", bufs=3) as stfpool,
            tc.tile_pool(name="misc", bufs=1) as mpool,
            tc.tile_pool(name="ps", bufs=8, space="PSUM") as pspool,
        ):
            # hsq resident: serves K/V shard projections and Q projection.
            hq = hpool.tile([128, HC, QPC], BF16, tag="hs")
            nc.sync.dma_start(out=hq[:], in_=hsq[:].rearrange("(c p) q -> p c q", p=128))

            # ---------- Phase B: KT shard (this core's QPC keys) ----------
            wk = wpool.tile([128, HC, H], BF16, tag="w")
            nc.sync.dma_start(out=wk[:], in_=wkT[:].rearrange("(c p) o -> p c o", p=128))
            for kcb in range(QPC // NH):
                for hb in range(HC):
                    ps = pspool.tile([128, NH], F32, tag="ps")
                    for hc in range(HC):
                        nc.tensor.matmul(
                            ps[:],
                            wk[:, hc, hb * 128:(hb + 1) * 128],
                            hq[:, hc, kcb * NH:(kcb + 1) * NH],
                            start=(hc == 0),
                            stop=(hc == HC - 1),
                        )
                    st = stpool.tile([128, NH], BF16, tag="stg")
                    nc.scalar.copy(st[:], ps[:])
                    nc.sync.dma_start(
                        out=kt_p[hb, :, kcb * NH:(kcb + 1) * NH], in_=st[:]
                    )

            # ---------- Phase C: V shard ----------
            wv = wpool.tile([128, HC, H], BF16, tag="w")
            nc.sync.dma_start(out=wv[:], in_=wvT[:].rearrange("(c p) o -> p c o", p=128))
            for kcb in range(QPC // NH):
                for kb4 in range(NH // 128):
                    for hh in range(H // NH):
                        ps = pspool.tile([128, NH], F32, tag="ps")
                        for hc in range(HC):
                            nc.tensor.matmul(
                                ps[:],
                                hq[:, hc, kcb * NH + kb4 * 128: kcb * NH + (kb4 + 1) * 128],
                                wv[:, hc, hh * NH:(hh + 1) * NH],
                                start=(hc == 0),
                                stop=(hc == HC - 1),
                            )
                        st = stpool.tile([128, NH], BF16, tag="stg")
                        nc.scalar.copy(st[:], ps[:])
                        nc.sync.dma_start(
                            out=v_p[
                                kcb * NH + kb4 * 128: kcb * NH + (kb4 + 1) * 128,
                                hh * NH:(hh + 1) * NH,
                            ],
                            in_=st[:],
                        )

            # ---------- AllGather K/V shards across the 4-core group ------
            nc.gpsimd.collective_compute(
                "AllGather",
                mybir.AluOpType.bypass,
                replica_groups=groups,
                ins=[kt_p[:]],
                outs=[kt_g[:]],
            )
            nc.gpsimd.collective_compute(
                "AllGather",
                mybir.AluOpType.bypass,
                replica_groups=groups,
                ins=[v_p[:]],
                outs=[v_g[:]],
            )

            # ---------- Phase A: QT projection (overlaps the collective) --
            wq = wpool.tile([128, HC, H], BF16, tag="w")
            nc.sync.dma_start(out=wq[:], in_=wqT[:].rearrange("(c p) o -> p c o", p=128))
            for qg in range(NQG):
                for hb in range(HC):
                    ps = pspool.tile([128, QR], F32, tag="ps")
                    for hc in range(HC):
                        nc.tensor.matmul(
                            ps[:],
                            wq[:, hc, hb * 128:(hb + 1) * 128],
                            hq[:, hc, qg * QR:(qg + 1) * QR],
                            start=(hc == 0),
                            stop=(hc == HC - 1),
                        )
                    st = stpool.tile([128, QR], BF16, tag="stg")
                    nc.scalar.copy(st[:], ps[:])
                    nc.sync.dma_start(
                        out=qt_d[hb, :, qg * QR:(qg + 1) * QR], in_=st[:]
                    )

            # ---------- Phase D: attention + out-proj ----------
            ones_col = mpool.tile([128, 1], BF16, tag="m_ones")
            nc.vector.memset(ones_col[:], 1.0)
            ones_row = mpool.tile([1, 128], F32, tag="m_onesr")
            nc.vector.memset(ones_row[:], 1.0)

            wo = wpool.tile([128, HC, H], BF16, tag="w")
            nc.sync.dma_start(out=wo[:], in_=woT[:].rearrange("(c p) o -> p c o", p=128))

            KL = 256  # keys per kt_g load
            for qg in range(NQG):
                qsl = slice(qg * QR, (qg + 1) * QR)
                qt = qtpool.tile([128, HC, QR], BF16, tag="qt")
                nc.sync.dma_start(
                    out=qt[:], in_=qt_d[:, :, qsl].rearrange("c p q -> p c q")
                )
                pt = ptpool.tile([128, KB, QR], BF16, tag="pt")
                l_ps = pspool.tile([1, QR], F32, tag="ps")

                def emit_l_mm(kb):
                    nc.tensor.matmul(
                        l_ps[:],
                        ones_col[:, 0:1],
                        pt[:, kb, :],
                        start=(kb == 0),
                        stop=(kb == KB - 1),
                    )

                # --- D1: S^T, P^T = exp(scale*S^T + mask^T), l ---
                for kbq in range(S // KL):
                    g, lo = (kbq * KL) // QPC, (kbq * KL) % QPC
                    ktq = spool.tile([128, HC, KL], BF16, tag="stream")
                    nc.sync.dma_start(
                        out=ktq[:],
                        in_=kt_g[g, :, :, lo:lo + KL].rearrange("c p k -> p c k"),
                    )
                    mk = mkpool.tile([128, KL // 128, QR], BF16, tag="mk")
                    nc.sync.dma_start(
                        out=mk[:],
                        in_=maskT[kbq * KL:(kbq + 1) * KL, qsl].rearrange(
                            "(b p) q -> p b q", p=128
                        ),
                    )
                    for kb4 in range(KL // 128):
                        kb = kbq * (KL // 128) + kb4
                        ps = pspool.tile([128, QR], F32, tag="ps")
                        for hc in range(HC):
                            nc.tensor.matmul(
                                ps[:],
                                ktq[:, hc, kb4 * 128:(kb4 + 1) * 128],
                                qt[:, hc, :],
                                start=(hc == 0),
                                stop=(hc == HC - 1),
                            )
                        tmp = stfpool.tile([128, QR], F32, tag="stgf")
                        nc.vector.scalar_tensor_tensor(
                            out=tmp[:],
                            in0=ps[:],
                            scalar=cfg.scale,
                            in1=mk[:, kb4, :],
                            op0=mm,
                            op1=mybir.AluOpType.add,
                        )
                        nc.scalar.activation(
                            out=pt[:, kb, :], in_=tmp[:],
                            func=mybir.ActivationFunctionType.Exp,
                        )
                        if kb >= 2:
                            emit_l_mm(kb - 2)
                for kb in range(KB - 2, KB):
                    emit_l_mm(kb)

                # --- D2: reciprocal + broadcast ---
                l_sb = mpool.tile([1, QR], F32, tag="m_l")
                nc.scalar.copy(l_sb[:], l_ps[:])
                rl = mpool.tile([1, QR], F32, tag="m_rl")
                nc.vector.reciprocal(rl[:], l_sb[:])
                b_ps = pspool.tile([128, QR], F32, tag="ps")
                nc.tensor.matmul(b_ps[:], ones_row[:], rl[:], start=True, stop=True)
                rb = mpool.tile([128, QR], F32, tag="m_rb")
                nc.scalar.copy(rb[:], b_ps[:])

                # --- D3: ctx^T with fused divide ---
                ct = ctpool.tile([128, HC, QR], BF16, tag="ct")
                for ho in range(HC // 8):
                    cps = [
                        pspool.tile([128, QR], F32, tag="ps", name=f"cps{i}")
                        for i in range(8)
                    ]
                    for kb in range(KB):
                        g, lr = (kb * 128) // QPC, (kb * 128) % QPC
                        vt = vpool.tile([128, 8, 128], BF16, tag="v")
                        nc.sync.dma_start(
                            out=vt[:],
                            in_=v_g[
                                g, lr:lr + 128, ho * 1024:(ho + 1) * 1024
                            ].rearrange("p (b h) -> p b h", b=8),
                        )
                        for i8 in range(8):
                            nc.tensor.matmul(
                                cps[i8][:],
                                vt[:, i8, :],
                                pt[:, kb, :],
                                start=(kb == 0),
                                stop=(kb == KB - 1),
                            )
                    for i8 in range(8):
                        nc.vector.tensor_mul(
                            out=ct[:, ho * 8 + i8, :], in0=cps[i8][:], in1=rb[:]
                        )

                # --- D4: output projection ---
                for qb in range(QR // 128):
                    for hh in range(H // NH):
                        ps = pspool.tile([128, NH], F32, tag="ps")
                        for hc in range(HC):
                            nc.tensor.matmul(
                                ps[:],
                                ct[:, hc, qb * 128:(qb + 1) * 128],
                                wo[:, hc, hh * NH:(hh + 1) * NH],
                                start=(hc == 0),
                                stop=(hc == HC - 1),
                            )
                        ob = stfpool.tile([128, NH], F32, tag="stgf")
                        nc.scalar.copy(ob[:], ps[:])
                        nc.sync.dma_start(
                            out=o_out[
                                qg * QR + qb * 128: qg * QR + (qb + 1) * 128,
                                hh * NH:(hh + 1) * NH,
                            ],
                            in_=ob[:],
                        )
    nc.finalize()
    return nc


def make_in_maps(cfg: Cfg, hidden_states, attention_mask, Wq, Wk, Wv, Wo, n_cores=8):
    """Build the 8 per-core input dicts (host-side prep: transpose + bf16)."""
    B = hidden_states.shape[0]
    gpc = n_cores // B  # cores per batch element
    wq_t = np.ascontiguousarray(Wq.T.astype(NP_BF16))
    wk_t = np.ascontiguousarray(Wk.T.astype(NP_BF16))
    wv_t = np.ascontiguousarray(Wv.T.astype(NP_BF16))
    wo_t = np.ascontiguousarray(Wo.T.astype(NP_BF16))
    in_maps = []
    for c in range(n_cores):
        b, g = c // gpc, c % gpc
        q0 = g * cfg.QPC
        hsq_c = np.ascontiguousarray(
            hidden_states[b, q0:q0 + cfg.QPC, :].T.astype(NP_BF16)
        )  # [H, QPC] — this core's query (= key-shard) columns
        msk = attention_mask[b, q0:q0 + cfg.QPC, :]  # [QPC, S]
        mskT_c = np.ascontiguousarray(msk.T.astype(NP_BF16))
        in_maps.append(
            {
                "hsq": hsq_c,
                "wqt": wq_t,
                "wkt": wk_t,
                "wvt": wv_t,
                "wot": wo_t,
                "maskt": mskT_c,
            }
        )
    return in_maps


def assemble_output(cfg: Cfg, results, B, S, H, n_cores=8):
    out = np.empty((B, S, H), dtype=np.float32)
    gpc = n_cores // B
    for c in range(n_cores):
        b, g = c // gpc, c % gpc
        out[b, g * cfg.QPC:(g + 1) * cfg.QPC, :] = results[c]["o"]
    return out


_CACHED_NC = None


def kernel(hidden_states, attention_mask, Wq, Wk, Wv, Wo, **kw):
    global _CACHED_NC
    B, S, H = hidden_states.shape
    cfg = Cfg(S=S, H=H, QPC=(B * S) // 8)
    if _CACHED_NC is None:
        _CACHED_NC = build_nc(cfg)
    nc = _CACHED_NC
    in_maps = make_in_maps(cfg, np.asarray(hidden_states), np.asarray(attention_mask),
                           np.asarray(Wq), np.asarray(Wk), np.asarray(Wv), np.asarray(Wo))
    core_ids = list(range(8))
    res = run_bass_kernel_spmd(nc, in_maps, core_ids)
    return assemble_output(cfg, res.results, B, S, H)


# revision 13
# speedup vs baseline: 8.2681x; 8.2665x over previous
"""Trainium2 Bass kernel for single-"head" LlamaAttention.

Reference computation (per batch b):
    q = hs @ Wq.T ; k = hs @ Wk.T ; v = hs @ Wv.T          # [S, H]
    scores = (q @ k.T) / sqrt(128) + mask                  # [S, S]
    probs  = softmax(scores, axis=-1)
    out    = (probs @ v) @ Wo.T                            # [S, H]

Shapes: B=2, S=4096, H=2048, fp32 I/O.

Sharding: 8 cores, 4 per batch element; each core owns 1024 query rows,
which are also its shard of 1024 keys.  Each core projects K/V only for its
own shard, then an AllGather within each 4-core group assembles the full
per-batch K^T and V — removing the 2x K/V-projection replication a
collective-free layout would need.  The Q projection is emitted after the
AllGather so PE work overlaps the collective.

Per-core pipeline (all matmuls bf16 with fp32 PSUM accumulation):
  B:   KT shard  = Wk proj of own columns           -> kt_p
  C:   V shard   = Wv proj                          -> v_p
  CC:  AllGather kt_p -> kt_g, v_p -> v_g  (groups [0-3], [4-7])
  A:   QT[h,q]   = Wq proj                          -> qt_d (DRAM scratch)
  D (per query half):
    D1:  S^T[k,q] = KT-chunks.T @ QT-chunks  (keys on partitions)
         P^T = exp(scale*S^T + mask^T)  (ACT), l[q] = ones.T @ P^T (PE)
    D2:  rl = 1/l (DVE), broadcast to 128 partitions via K=1 matmul
    D3:  ctx^T[h,q] = sum_k V-tiles.T @ P^T, multiplied by rl on PSUM->SBUF
    D4:  O[q,h] = ctx^T-chunks.T @ WoT  -> output
"""

import math
import os
import sys

import numpy as np

sys.path.insert(0, "/opt/trn_rl_repo")

import ml_dtypes  # noqa: E402

import concourse.bass as bass  # noqa: E402
import concourse.tile as tile  # noqa: E402
from concourse import bacc, mybir  # noqa: E402
from concourse.bass_utils import run_bass_kernel_spmd  # noqa: E402

BF16 = mybir.dt.bfloat16
F32 = mybir.dt.float32
NP_BF16 = ml_dtypes.bfloat16


class Cfg:
    def __init__(self, S=4096, H=2048, QPC=1024, head_dim=128):
        self.S = S          # keys per batch
        self.H = H          # hidden
        self.QPC = QPC      # queries per core (also keys per core shard)
        self.GPC = 4        # cores per batch group
        self.HC = H // 128  # contraction chunks
        self.KB = S // 128  # key blocks
        self.NH = 512       # free-dim tile
        self.QR = min(QPC, 512)  # queries per round
        self.NQG = QPC // self.QR
        self.scale = 1.0 / math.sqrt(head_dim)
        assert QPC % self.NH == 0 and S == self.GPC * QPC


def build_nc(cfg: Cfg) -> bass.Bass:
    S, H, QPC = cfg.S, cfg.H, cfg.QPC
    HC, KB, NH, QR, NQG, GPC = cfg.HC, cfg.KB, cfg.NH, cfg.QR, cfg.NQG, cfg.GPC

    nc = bacc.Bacc(None, target_bir_lowering=False, num_devices=2 * GPC)

    hsq = nc.dram_tensor("hsq", [H, QPC], BF16, kind="ExternalInput")
    wqT = nc.dram_tensor("wqt", [H, H], BF16, kind="ExternalInput")
    wkT = nc.dram_tensor("wkt", [H, H], BF16, kind="ExternalInput")
    wvT = nc.dram_tensor("wvt", [H, H], BF16, kind="ExternalInput")
    woT = nc.dram_tensor("wot", [H, H], BF16, kind="ExternalInput")
    maskT = nc.dram_tensor("maskt", [S, QPC], BF16, kind="ExternalInput")
    o_out = nc.dram_tensor("o", [QPC, H], F32, kind="ExternalOutput")

    qt_d = nc.dram_tensor("qt_d", [HC, 128, QPC], BF16)
    kt_p = nc.dram_tensor("kt_p", [HC, 128, QPC], BF16)
    v_p = nc.dram_tensor("v_p", [QPC, H], BF16)
    kt_g = nc.dram_tensor("kt_g", [GPC, HC, 128, QPC], BF16)
    v_g = nc.dram_tensor("v_g", [GPC, QPC, H], BF16)

    groups = [list(range(g * GPC, (g + 1) * GPC)) for g in range(2)]
    mm = mybir.AluOpType.mult

    with tile.TileContext(nc) as tc:
        with (
            tc.tile_pool(name="hs", bufs=1) as hpool,
            tc.tile_pool(name="w", bufs=1) as wpool,
            tc.tile_pool(name="stream", bufs=3) as spool,
            tc.tile_pool(name="qt", bufs=1) as qtpool,
            tc.tile_pool(name="pt", bufs=1) as ptpool,
            tc.tile_pool(name="ct", bufs=1) as ctpool,
            tc.tile_pool(name="mk", bufs=1) as mkpool,
            tc.tile_pool(name="v", bufs=2) as vpool,
            tc.tile_pool(name="stg", bufs=3) as stpool,
            tc.tile_pool(name="stgf", bufs=3) as stfpool,
            tc.tile_pool(name="misc", bufs=1) as mpool,
            tc.tile_pool(name="ps", bufs=8, space="PSUM") as pspool,
        ):
            # hsq resident: serves K/V shard projections and Q projection.
            hq = hpool.tile([128, HC, QPC], BF16, tag="hs")
            nc.sync.dma_start(out=hq[:], in_=hsq[:].rearrange("(c p) q -> p c q", p=128))

            # ---------- Phase B: KT shard (this core's QPC keys) ----------
            wk = wpool.tile([128, HC, H], BF16, tag="w")
            nc.sync.dma_start(out=wk[:], in_=wkT[:].rearrange("(c p) o -> p c o", p=128))
            for kcb in range(QPC // NH):
                for hb in range(HC):
                    ps = pspool.tile([128, NH], F32, tag="ps")
                    for hc in range(HC):
                        nc.tensor.matmul(
                            ps[:],
                            wk[:, hc, hb * 128:(hb + 1) * 128],
                            hq[:, hc, kcb * NH:(kcb + 1) * NH],
                            start=(hc == 0),
                            stop=(hc == HC - 1),
                        )
                    st = stpool.tile([128, NH], BF16, tag="stg")
                    nc.scalar.copy(st[:], ps[:])
                    nc.sync.dma_start(
                        out=kt_p[hb, :, kcb * NH:(kcb + 1) * NH], in_=st[:]
                    )

            # ---------- Phase C: V shard ----------
            wv = wpool.tile([128, HC, H], BF16, tag="w")
            nc.sync.dma_start(out=wv[:], in_=wvT[:].rearrange("(c p) o -> p c o", p=128))
            for kcb in range(QPC // NH):
                for kb4 in range(NH // 128):
                    for hh in range(H // NH):
                        ps = pspool.tile([128, NH], F32, tag="ps")
                        for hc in range(HC):
                            nc.tensor.matmul(
                                ps[:],
                                hq[:, hc, kcb * NH + kb4 * 128: kcb * NH + (kb4 + 1) * 128],
                                wv[:, hc, hh * NH:(hh + 1) * NH],
                                start=(hc == 0),
                                stop=(hc == HC - 1),
                            )
                        st = stpool.tile([128, NH], BF16, tag="stg")
                        nc.scalar.copy(st[:], ps[:])
                        nc.sync.dma_start(
                            out=v_p[
                                kcb * NH + kb4 * 128: kcb * NH + (kb4 + 1) * 128,
                                hh * NH:(hh + 1) * NH,
                            ],
                            in_=st[:],
                        )

            # ---------- AllGather K/V shards across the 4-core group ------
            nc.gpsimd.collective_compute(
                "AllGather",
                mybir.AluOpType.bypass,
                replica_groups=groups,
                ins=[kt_p[:]],
                outs=[kt_g[:]],
            )
            nc.gpsimd.collective_compute(
                "AllGather",
                mybir.AluOpType.bypass,
                replica_groups=groups,
                ins=[v_p[:]],
                outs=[v_g[:]],
            )

            # ---------- Phase A: QT projection (overlaps the collective) --
            wq = wpool.tile([128, HC, H], BF16, tag="w")
            nc.sync.dma_start(out=wq[:], in_=wqT[:].rearrange("(c p) o -> p c o", p=128))
            for qg in range(NQG):
                for hb in range(HC):
                    ps = pspool.tile([128, QR], F32, tag="ps")
                    for hc in range(HC):
                        nc.tensor.matmul(
                            ps[:],
                            wq[:, hc, hb * 128:(hb + 1) * 128],
                            hq[:, hc, qg * QR:(qg + 1) * QR],
                            start=(hc == 0),
                            stop=(hc == HC - 1),
                        )
                    st = stpool.tile([128, QR], BF16, tag="stg")
                    nc.scalar.copy(st[:], ps[:])
                    nc.sync.dma_start(
                        out=qt_d[hb, :, qg * QR:(qg + 1) * QR], in_=st[:]
                    )

            # ---------- Phase D: attention + out-proj ----------
            ones_col = mpool.tile([128, 1], BF16, tag="m_ones")
            nc.vector.memset(ones_col[:], 1.0)
            ones_row = mpool.tile([1, 128], F32, tag="m_onesr")
            nc.vector.memset(ones_row[:], 1.0)

            wo = wpool.tile([128, HC, H], BF16, tag="w")
            nc.sync.dma_start(out=wo[:], in_=woT[:].rearrange("(c p) o -> p c o", p=128))

            KL = 256  # keys per kt_g load
            for qg in range(NQG):
                qsl = slice(qg * QR, (qg + 1) * QR)
                qt = qtpool.tile([128, HC, QR], BF16, tag="qt")
                nc.sync.dma_start(
                    out=qt[:], in_=qt_d[:, :, qsl].rearrange("c p q -> p c q")
                )
                pt = ptpool.tile([128, KB, QR], BF16, tag="pt")
                l_ps = pspool.tile([1, QR], F32, tag="ps")

                def emit_l_mm(kb):
                    nc.tensor.matmul(
                        l_ps[:],
                        ones_col[:, 0:1],
                        pt[:, kb, :],
                        start=(kb == 0),
                        stop=(kb == KB - 1),
                    )

                # --- D1: S^T, P^T = exp(scale*S^T + mask^T), l ---
                for kbq in range(S // KL):
                    g, lo = (kbq * KL) // QPC, (kbq * KL) % QPC
                    ktq = spool.tile([128, HC, KL], BF16, tag="stream")
                    nc.sync.dma_start(
                        out=ktq[:],
                        in_=kt_g[g, :, :, lo:lo + KL].rearrange("c p k -> p c k"),
                    )
                    mk = mkpool.tile([128, KL // 128, QR], BF16, tag="mk")
                    nc.sync.dma_start(
                        out=mk[:],
                        in_=maskT[kbq * KL:(kbq + 1) * KL, qsl].rearrange(
                            "(b p) q -> p b q", p=128
                        ),
                    )
                    for kb4 in range(KL // 128):
                        kb = kbq * (KL // 128) + kb4
                        ps = pspool.tile([128, QR], F32, tag="ps")
                        for hc in range(HC):
                            nc.tensor.matmul(
                                ps[:],
                                ktq[:, hc, kb4 * 128:(kb4 + 1) * 128],
                                qt[:, hc, :],
                                start=(hc == 0),
                                stop=(hc == HC - 1),
                            )
                        tmp = stfpool.tile([128, QR], F32, tag="stgf")
                        nc.vector.scalar_tensor_tensor(
                            out=tmp[:],
                            in0=ps[:],
                            scalar=cfg.scale,
                            in1=mk[:, kb4, :],
                            op0=mm,
                            op1=mybir.AluOpType.add,
                        )
                        nc.scalar.activation(
                            out=pt[:, kb, :], in_=tmp[:],
                            func=mybir.ActivationFunctionType.Exp,
                        )
                        if kb >= 2:
                            emit_l_mm(kb - 2)
                for kb in range(KB - 2, KB):
                    emit_l_mm(kb)

                # --- D2: reciprocal + broadcast ---
                l_sb = mpool.tile([1, QR], F32, tag="m_l")
                nc.scalar.copy(l_sb[:], l_ps[:])
                rl = mpool.tile([1, QR], F32, tag="m_rl")
                nc.vector.reciprocal(rl[:], l_sb[:])
                b_ps = pspool.tile([128, QR], F32, tag="ps")
                nc.tensor.matmul(b_ps[:], ones_row[:], rl[:], start=True, stop=True)
                rb = mpool.tile([128, QR], F32, tag="m_rb")
                nc.scalar.copy(rb[:], b_ps[:])

                # --- D3: ctx^T with fused divide ---
                ct = ctpool.tile([128, HC, QR], BF16, tag="ct")
                for ho in range(HC // 8):
                    cps = [
                        pspool.tile([128, QR], F32, tag="ps", name=f"cps{i}")
                        for i in range(8)
                    ]
                    for kb in range(KB):
                        g, lr = (kb * 128) // QPC, (kb * 128) % QPC
                        vt = vpool.tile([128, 8, 128], BF16, tag="v")
                        nc.sync.dma_start(
                            out=vt[:],
                            in_=v_g[
                                g, lr:lr + 128, ho * 1024:(ho + 1) * 1024
                            ].rearrange("p (b h) -> p b h", b=8),
                        )
                        for i8 in range(8):
                            nc.tensor.matmul(
                                cps[i8][:],
                                vt[:, i8, :],
                                pt[:, kb, :],
                                start=(kb == 0),
                                stop=(kb == KB - 1),
                            )
                    for i8 in range(8):
                        nc.vector.tensor_mul(
                            out=ct[:, ho * 8 + i8, :], in0=cps[i8][:], in1=rb[:]
                        )

                # --- D4: output projection ---
                for qb in range(QR // 128):
                    for hh in range(H // NH):
                        ps = pspool.tile([128, NH], F32, tag="ps")
                        for hc in range(HC):
                            nc.tensor.matmul(
                                ps[:],
                                ct[:, hc, qb * 128:(qb + 1) * 128],
                                wo[:, hc, hh * NH:(hh + 1) * NH],
                                start=(hc == 0),
                                stop=(hc == HC - 1),
                            )
                        ob = stfpool.tile([128, NH], F32, tag="stgf")
                        nc.scalar.copy(ob[:], ps[:])
                        nc.sync.dma_start(
                            out=o_out[
                                qg * QR + qb * 128: qg * QR + (qb + 1) * 128,
                                hh * NH:(hh + 1) * NH,
                            ],
                            in_=ob[:],
                        )
    nc.finalize()
    return nc


def make_in_maps(cfg: Cfg, hidden_states, attention_mask, Wq, Wk, Wv, Wo, n_cores=8):
    """Build the 8 per-core input dicts (host-side prep: transpose + bf16)."""
    B = hidden_states.shape[0]
    gpc = n_cores // B  # cores per batch element
    wq_t = np.ascontiguousarray(Wq.T.astype(NP_BF16))
    wk_t = np.ascontiguousarray(Wk.T.astype(NP_BF16))
    wv_t = np.ascontiguousarray(Wv.T.astype(NP_BF16))
    wo_t = np.ascontiguousarray(Wo.T.astype(NP_BF16))
    in_maps = []
    for c in range(n_cores):
        b, g = c // gpc, c % gpc
        q0 = g * cfg.QPC
        hsq_c = np.ascontiguousarray(
            hidden_states[b, q0:q0 + cfg.QPC, :].T.astype(NP_BF16)
        )  # [H, QPC] — this core's query (= key-shard) columns
        msk = attention_mask[b, q0:q0 + cfg.QPC, :]  # [QPC, S]
        mskT_c = np.ascontiguousarray(msk.T.astype(NP_BF16))
        in_maps.append(
            {
                "hsq": hsq_c,
                "wqt": wq_t,
                "wkt": wk_t,
                "wvt": wv_t,
                "wot": wo_t,
                "maskt": mskT_c,
            }
        )
    return in_maps


def assemble_output(cfg: Cfg, results, B, S, H, n_cores=8):
    out = np.empty((B, S, H), dtype=np.float32)
    gpc = n_cores // B
    for c in range(n_cores):
        b, g = c // gpc, c % gpc
        out[b, g * cfg.QPC:(g + 1) * cfg.QPC, :] = results[c]["o"]
    return out


_CACHED_NC = None


def kernel(hidden_states, attention_mask, Wq, Wk, Wv, Wo, **kw):
    global _CACHED_NC
    B, S, H = hidden_states.shape
    cfg = Cfg(S=S, H=H, QPC=(B * S) // 8)
    if _CACHED_NC is None:
        _CACHED_NC = build_nc(cfg)
    nc = _CACHED_NC
    in_maps = make_in_maps(cfg, np.asarray(hidden_states), np.asarray(attention_mask),
                           np.asarray(Wq), np.asarray(Wk), np.asarray(Wv), np.asarray(Wo))
    core_ids = list(range(8))
    last_exc = None
    for _ in range(3):  # the axon tunnel occasionally drops a worker
        try:
            res = run_bass_kernel_spmd(nc, in_maps, core_ids)
            return assemble_output(cfg, res.results, B, S, H)
        except Exception as e:  # noqa: BLE001
            last_exc = e
    raise last_exc
